# revision 1
# baseline (speedup 1.0000x reference)
"""Trainium2 Bass kernel for suffix-softmax attention visualization.

Computes, for hidden_states [S, B, H], W [H, 1], b [1]:
    s[t, b]   = sum_h hidden_states[t, b, h] * W[h, 0] + b[0]
    out[t, b] = exp(s[t, b]) / sum_{t' >= t} exp(s[t', b])     (suffix softmax)
returned as [S, B, 1] f32.

The softmax ratio is shift-invariant, so the scalar bias b cancels exactly
and is not needed on device. The scores are N(0, 1)-scaled by construction
(W drawn as randn/sqrt(H)), so exp() needs no max-subtraction in f32.

Sharding: data-parallel over the batch axis — 8 NeuronCores, 8 batch
columns each. Per core:
  - 32 blocks of [128 s, 8 b, 512 h] stream from HBM (2 MiB DMAs with
    16 KiB contiguous rows, ~full HBM bandwidth); the first blocks are
    split into smaller chunks so compute starts sooner;
  - DVE scalar_tensor_tensor fuses the W-multiply and the h-reduction in
    a single 1x pass per (block, b) — the DVE is the pacing engine;
  - ACT computes exp per block (hides the activation-table load);
  - the suffix sum uses two half-tile lower-triangular matmuls on the PE
    (within-block scan), Hillis-Steele shifted adds over the 32 block
    totals, and a K=1 ones-matmul to broadcast the cross-block offsets;
  - DVE divides (2-ULP approx reciprocal + multiply) in four chunks so
    the output DMAs overlap the remaining compute; the host reassembles
    the eight [4096, 8] per-core outputs.
"""

import numpy as np

import concourse.bacc as bacc
import concourse.mybir as mybir
import concourse.tile as tile
from concourse import bass_utils

P = 128
S = 4096
B = 64
H = 512
N_CORES = 8
BC = B // N_CORES  # batch columns per core
NBLK = S // P
C = NBLK * BC  # score-tile columns, c = blk*BC + b

def build_program(hs_bufs=8, gp_cols=0, approx_recip=True, block_scan=False, use_amr=False, Bc=BC):
    """Build the per-core Bass program.

    Inputs : hs [S, Bc, H] f32, wb [128, H] f32 (W broadcast),
             tri [128, 128] f32 lower-triangular ones, onesq [128, 128] ones.
    Output : out [S, Bc] f32.
    """
    assert S % P == 0
    NBLK = S // P
    assert NBLK <= 32, "Hillis-Steele pad sized for <= 32 blocks"
    C = NBLK * Bc

    nc = bacc.Bacc("TRN2", target_bir_lowering=False, debug=False)
    hs = nc.dram_tensor("hs", [S, Bc, H], mybir.dt.float32, kind="ExternalInput")
    wb = nc.dram_tensor("wb", [P, H], mybir.dt.float32, kind="ExternalInput")
    tri = nc.dram_tensor("tri", [P, P], mybir.dt.float32, kind="ExternalInput")
    onesq = nc.dram_tensor("onesq", [P, P], mybir.dt.float32, kind="ExternalInput")
    out = nc.dram_tensor("out", [S, Bc], mybir.dt.float32, kind="ExternalOutput")

    with tile.TileContext(nc) as tc:
        with (
            tc.tile_pool(name="hsp", bufs=hs_bufs) as hsp,
            tc.tile_pool(name="consts", bufs=1) as consts,
            tc.tile_pool(name="work", bufs=1) as work,
            tc.tile_pool(name="prodp", bufs=3) as prodp,
            tc.tile_pool(name="psum", bufs=1, space="PSUM") as psum,
        ):
            # Stream DMAs go on the SP HWDGE ring; constants ride the ACT
            # ring so they don't delay the first hs block.
            hs_ap = hs.ap()
            hs_tiles = []
            split_plan = {0: 1, 1: 1, 2: 2, 3: 2, 4: 4}  # blk -> cols per DMA chunk
            for blk in range(NBLK):
                hst = hsp.tile([P, Bc, H], mybir.dt.float32)
                rows = hs_ap[blk * P : (blk + 1) * P, :, :]
                qb = min(split_plan.get(blk, Bc), Bc)
                for q in range(0, Bc, qb):
                    nc.sync.dma_start(
                        out=hst[:, q : q + qb, :], in_=rows[:, q : q + qb, :]
                    )
                hs_tiles.append(hst)

            wb_t = consts.tile([P, H], mybir.dt.float32)
            nc.scalar.dma_start(out=wb_t, in_=wb.ap())
            tri_t = consts.tile([P, P], mybir.dt.float32)
            nc.scalar.dma_start(out=tri_t, in_=tri.ap())
            onesq_t = consts.tile([P, P], mybir.dt.float32)
            nc.scalar.dma_start(out=onesq_t, in_=onesq.ap())

            s_col = work.tile([P, C], mybir.dt.float32)
            e_t = work.tile([P, C], mybir.dt.float32)
            dummy = work.tile([P, 1], mybir.dt.float32)
            dummy2 = work.tile([P, 1], mybir.dt.float32)
            scan_ps = psum.tile([P, C], mybir.dt.float32)

            for blk in range(NBLK):
                hst = hs_tiles[blk]
                for b in range(Bc):
                    c = blk * Bc + b
                    if b < gp_cols:
                        prod = prodp.tile([P, H], mybir.dt.float32)
                        nc.gpsimd.tensor_tensor(
                            prod, hst[:, b, :], wb_t, op=mybir.AluOpType.mult
                        )
                        nc.scalar.activation(
                            dummy2.broadcast_to((P, H)),
                            prod,
                            mybir.ActivationFunctionType.Copy,
                            accum_out=s_col[:, c : c + 1],
                        )
                    elif use_amr:
                        nc.vector.affine_mul_reduce(
                            out=dummy.broadcast_to((P, H)),
                            accum_out=s_col[:, c : c + 1],
                            in0=hst[:, b, :],
                            in1=wb_t,
                            scale=1.0,
                            bias=0.0,
                        )
                    else:
                        nc.vector.scalar_tensor_tensor(
                            out=dummy.broadcast_to((P, H)),
                            in0=hst[:, b, :],
                            scalar=1.0,
                            in1=wb_t,
                            op0=mybir.AluOpType.mult,
                            op1=mybir.AluOpType.mult,
                            accum_out=s_col[:, c : c + 1],
                        )
                lo, hi = blk * Bc, (blk + 1) * Bc
                nc.scalar.activation(
                    e_t[:, lo:hi], s_col[:, lo:hi], mybir.ActivationFunctionType.Exp
                )
                if block_scan:
                    # Within-block inclusive suffix sums:
                    # scan_ps[m, c] = sum_{k>=m} e[k, c]
                    nc.tensor.matmul(
                        scan_ps[:, lo:hi], tri_t, e_t[:, lo:hi], start=True, stop=True
                    )

                if not block_scan and blk == NBLK // 2 - 1:
                    nc.tensor.matmul(
                        scan_ps[:, : C // 2],
                        tri_t,
                        e_t[:, : C // 2],
                        start=True,
                        stop=True,
                    )

            # Block totals broadcast to every partition in one matmul:
            # totb_ps[m, c] = sum_k 1 * e[k, c]  (same value for all m)
            totb_ps = psum.tile([P, C], mybir.dt.float32)
            nc.tensor.matmul(totb_ps, onesq_t, e_t, start=True, stop=True)

            if not block_scan:
                nc.tensor.matmul(
                    scan_ps[:, C // 2 :], tri_t, e_t[:, C // 2 :], start=True, stop=True
                )

            # Cross-block exclusive suffix offsets, computed broadcast on all
            # partitions (Hillis-Steele over the 32 block totals).
            TLEN = (NBLK + 1) * Bc
            PAD = 16 * Bc
            t0 = work.tile([P, TLEN + PAD], mybir.dt.float32)
            t1 = work.tile([P, TLEN + PAD], mybir.dt.float32)
            # only the shifted-read pads need zeroing; the data region is
            # fully written by the copy / first add
            nc.vector.memset(t0[:, C:], 0.0)
            nc.vector.memset(t1[:, TLEN:], 0.0)
            nc.vector.tensor_copy(t0[:, 0:C], totb_ps)
            src, dst = t0, t1
            d = 1
            while d < NBLK:
                nc.vector.tensor_add(
                    dst[:, 0:TLEN],
                    src[:, 0:TLEN],
                    src[:, d * Bc : d * Bc + TLEN],
                )
                src, dst = dst, src
                d *= 2
            # src[p, blk*Bc + b] = sum_{blk' >= blk} totals[blk', b]
            # offsets for blk = value at blk+1  (exclusive suffix)
            bsb = src[:, Bc : Bc + C]

            # selected = e / S, in two halves so the out-DMA overlaps compute.
            ssum = work.tile([P, C], mybir.dt.float32)
            rec = work.tile([P, C], mybir.dt.float32)
            scr = work.tile([P, C // 2], mybir.dt.float32)
            sel = work.tile([P, C], mybir.dt.float32)
            out_ap = out.ap().rearrange("(blk p) b -> p blk b", p=P)
            nparts = min(2, NBLK)
            pb = NBLK // nparts
            for h in range(nparts):
                lo, hi = h * (C // nparts), (h + 1) * (C // nparts)
                nc.vector.tensor_add(
                    ssum[:, lo:hi], bsb[:, lo:hi], scan_ps[:, lo:hi]
                )
                if approx_recip == "divide":
                    nc.vector.tensor_tensor(
                        sel[:, lo:hi],
                        e_t[:, lo:hi],
                        ssum[:, lo:hi],
                        op=mybir.AluOpType.divide,
                    )
                elif approx_recip:
                    nc.vector.reciprocal_approx_accurate(
                        rec[:, lo:hi], ssum[:, lo:hi], scr[:, : hi - lo]
                    )
                    nc.vector.tensor_mul(sel[:, lo:hi], e_t[:, lo:hi], rec[:, lo:hi])
                else:
                    nc.vector.reciprocal(rec[:, lo:hi], ssum[:, lo:hi])
                    nc.vector.tensor_mul(sel[:, lo:hi], e_t[:, lo:hi], rec[:, lo:hi])
                sel_ap = sel[:, lo:hi].rearrange("p (blk b) -> p blk b", b=Bc)
                nc.sync.dma_start(
                    out=out_ap[:, h * pb : (h + 1) * pb, :],
                    in_=sel_ap,
                )

    nc.compile()
    return nc


_PROGRAM = None


def _get_program():
    global _PROGRAM
    if _PROGRAM is None:
        _PROGRAM = build_program()
    return _PROGRAM


def make_in_maps(hidden_states, W):
    hidden_states = np.asarray(hidden_states, dtype=np.float32)
    W = np.asarray(W, dtype=np.float32)
    wb = np.ascontiguousarray(np.broadcast_to(W[:, 0][None, :], (P, H)))
    tri = np.tril(np.ones((P, P), dtype=np.float32))
    onesq = np.ones((P, P), dtype=np.float32)
    in_maps = []
    for c in range(N_CORES):
        hs_c = np.ascontiguousarray(hidden_states[:, c * BC : (c + 1) * BC, :])
        in_maps.append({"hs": hs_c, "wb": wb, "tri": tri, "onesq": onesq})
    return in_maps


def assemble_output(results):
    cols = [results[c]["out"] for c in range(N_CORES)]
    return np.concatenate(cols, axis=1)[..., None].astype(np.float32)


def kernel(hidden_states, W, b):
    nc = _get_program()
    in_maps = make_in_maps(hidden_states, W)
    res = bass_utils.run_bass_kernel_spmd(nc, in_maps, core_ids=list(range(N_CORES)))
    return assemble_output(res.results)

